# revision 1
# baseline (speedup 1.0000x reference)
"""Trainium2 Bass kernel for nn_External_attention_44976897524182.

Math (folded on host):
    A  = lin0_w @ conv1_w            (64 x 128)
    ab = lin0_w @ conv1_b            (64,)
    Bs = (bn_scale * conv2_w) @ lin1_w   (128 x 64)
    shift = bn_beta - bn_mean * bn_scale
  With e = exp(A@x + ab), S[k] = sum_n e[k,n] (global over n),
    denom[n] = sum_k e[k,n]/S[k]
    out[c,n] = relu( (Bs @ (e/S))[c,n] / denom[n] + shift[c] + x[c,n] )

Sharding: 8 cores = 2 batches x 4 n-slices of 32768. Cross-core dep is S[k]
(64 floats): AllGather over groups [[0-3],[4-7]] + local sum.

v2 layout / schedule:
  - All HBM IO in bf16. n-major streams (xst residual, out) are pair-packed:
    DRAM rows hold 2 consecutive positions (256 ch = 512B) so DMA descriptors
    stay at the 512B line-rate threshold.
  - Phase 1: x (c-major bf16) tiles of 2048 cols; 4 bf16 matmuls per tile into
    a 2-bank PSUM (paired partition halves); one Exp activation per tile
    (FD=1024) emits bf16 e_sb and per-tile S partials via accum_out.
  - e_sb resident (128, 16384) bf16, paired: partitions 0-63 even 512-tiles,
    64-127 odd.
  - xst (residual, n-major pair-packed bf16) fully resident in SBUF
    (64KB/partition); prefetched during phase 1 tail + AllGather.
  - Phase 2 over 128 blocks of 256 positions = 2 chunks (even/odd positions,
    stride-2 lhsT column APs). Groups of 3 blocks (6 chunks) share a 2-bank
    PSUM tile; one strided reciprocal per group covers the 6 denominators.
    Normalize+residual: most groups do a batched ACT drain (PSUM->bf16 SBUF)
    then per-chunk gpsimd scalar_tensor_tensor (zz*r + xst); every Nth group
    uses direct DVE stt from PSUM. Relu: one DVE tensor_scalar_max (4x mode)
    per group, in place. Out store: one pair-packed DMA per group.
"""

import numpy as np

_B, _C = 2, 128
_D, _H, _W = 32, 64, 64
_N = _D * _H * _W          # 131072
_NCORES = 8
_SLICES = 4
_NSH = _N // _SLICES       # 32768 per core
_K = 64
_T = 512
_NSP = 16                  # phase-1 super-tiles of 2048 cols
_NBLK = _NSH // 256        # 128 blocks of 256 positions
_GB = 3                    # blocks per phase-2 group
_BN_EPS = 1e-5

# phase-2 group-type rotation: 2=P (ACT drain + Pool normalize_recip),
# 1=A (ACT scale-copy + DVE add), 0=D (DVE direct stt)
_TYPE_PATTERN = [2, 2, 1, 2, 2, 1, 2, 2, 1, 0]

_nc_cache = None
last_results = None


def _build(nsh=None, reps=1):
    global _nc_cache
    if nsh is None:
        nsh = _NSH
    full = nsh == _NSH and reps == 1
    if full and _nc_cache is not None:
        return _nc_cache

    from contextlib import ExitStack
    import concourse.bass as bass  # noqa: F401
    import concourse.bacc as bacc
    import concourse.tile as tile
    import concourse.mybir as mybir

    f32 = mybir.dt.float32
    bf16 = mybir.dt.bfloat16
    AF = mybir.ActivationFunctionType
    ALU = mybir.AluOpType

    nc = bacc.Bacc(
        trn_type="TRN2",
        target_bir_lowering=False,
        debug=False,
        num_devices=_NCORES,
    )
    NSH = nsh
    x_d = nc.dram_tensor("x", [_C, NSH], bf16, kind="ExternalInput").ap()
    xst_d = nc.dram_tensor("xst", [NSH // 2, 256], bf16, kind="ExternalInput").ap()
    at_d = nc.dram_tensor("a_t", [_C, _K], bf16, kind="ExternalInput").ap()
    bt_d = nc.dram_tensor("b_t", [_C, _C], bf16, kind="ExternalInput").ap()
    ab_d = nc.dram_tensor("ab2", [_C, 1], f32, kind="ExternalInput").ap()
    out_d = nc.dram_tensor("out", [NSH // 2, 256], bf16, kind="ExternalOutput").ap()

    with tile.TileContext(nc) as tc, ExitStack() as ctx:
        consts = ctx.enter_context(tc.tile_pool(name="consts", bufs=1))
        xpool = ctx.enter_context(tc.tile_pool(name="xpool", bufs=3))
        stp = ctx.enter_context(tc.tile_pool(name="stp", bufs=6))
        drp = ctx.enter_context(tc.tile_pool(name="drp", bufs=6))
        rp = ctx.enter_context(tc.tile_pool(name="rp", bufs=6))
        ps1 = ctx.enter_context(tc.tile_pool(name="ps1", bufs=2, space="PSUM"))
        ps2 = ctx.enter_context(tc.tile_pool(name="ps2", bufs=1, space="PSUM"))
        dram = ctx.enter_context(tc.tile_pool(name="dram", bufs=1, space="DRAM"))

        A_T = consts.tile([_C, _K], bf16)
        nc.sync.dma_start(out=A_T, in_=at_d)
        B_T = consts.tile([_C, _C], bf16)   # Bs^T duplicated into both halves
        nc.sync.dma_start(out=B_T, in_=bt_d)
        ab2 = consts.tile([_C, 1], f32)
        nc.sync.dma_start(out=ab2, in_=ab_d)

        e_sb = consts.tile([_C, NSH // 2], bf16)      # paired exp values
        xst_sb = consts.tile([_C, NSH // 256, 256], bf16)  # resident residual

        for _rep in range(reps):
            _emit_body(nc, tc, mybir, f32, bf16, AF, ALU, NSH,
                       x_d, xst_d, out_d, A_T, B_T, ab2, e_sb, xst_sb,
                       consts, xpool, stp, drp, rp, ps1, ps2, dram)

    nc.finalize()
    if full:
        _nc_cache = nc
    return nc


def _emit_body(nc, tc, mybir, f32, bf16, AF, ALU, NSH,
               x_d, xst_d, out_d, A_T, B_T, ab2, e_sb, xst_sb,
               consts, xpool, stp, drp, rp, ps1, ps2, dram):
    import concourse.bass as _bass
    NSP = NSH // 2048
    NBLK = NSH // 256

    # ---- phase 1: e = exp(A@x + ab) in bf16, accumulate S partials ----
    NP = NSH // 2048
    spart = consts.tile([_C, NP], f32, tag="spart")
    for P8 in range(NSH // 4096):
        xt = xpool.tile([_C, 4096], bf16, tag="xt")
        nc.sync.dma_start(out=xt, in_=x_d[:, P8 * 4096:(P8 + 1) * 4096])
        for h in range(2):
            P = 2 * P8 + h
            xo = 2048 * h
            pp = ps1.tile([_C, 1024], f32, tag="pp")
            nc.tensor.matmul(pp[0:_K, 0:512], lhsT=A_T,
                             rhs=xt[:, xo:xo + 512], start=True, stop=True)
            nc.tensor.matmul(pp[_K:_C, 0:512], lhsT=A_T,
                             rhs=xt[:, xo + 512:xo + 1024],
                             start=True, stop=True, tile_position=(0, _K))
            nc.tensor.matmul(pp[0:_K, 512:1024], lhsT=A_T,
                             rhs=xt[:, xo + 1024:xo + 1536],
                             start=True, stop=True)
            nc.tensor.matmul(pp[_K:_C, 512:1024], lhsT=A_T,
                             rhs=xt[:, xo + 1536:xo + 2048],
                             start=True, stop=True, tile_position=(0, _K))
            nc.scalar.activation(out=e_sb[:, P * 1024:(P + 1) * 1024], in_=pp,
                                 func=AF.Exp, bias=ab2, scale=1.0,
                                 accum_out=spart[:, P:P + 1])

    # ---- S: reduce partials, AllReduce the full 128-row partial (both
    # k-halves stacked; the half-fold happens after), recip ----
    sred = consts.tile([_C, 1], f32)
    nc.vector.tensor_reduce(out=sred, in_=spart,
                            axis=mybir.AxisListType.X, op=ALU.add)
    cc_in = dram.tile([_C, 1], f32)
    cc_out = dram.tile([_C, 1], f32)
    nc.sync.dma_start(out=cc_in, in_=sred)
    nc.gpsimd.collective_compute(
        "AllReduce", ALU.add,
        replica_groups=[[0, 1, 2, 3], [4, 5, 6, 7]],
        ins=[cc_in.opt()], outs=[cc_out.opt()])

    # ---- xst prefetch: emitted after the collective chain; tile_wait_until
    # keeps the 8 loads off the DMA rings until the phase-1 x stream ends,
    # so they run under the collective ----
    with tc.tile_wait_until(0.0235):
        for g2 in range(8):
            nc.sync.dma_start(
                out=xst_sb[:, (NBLK // 8) * g2:(NBLK // 8) * (g2 + 1), :],
                in_=xst_d[(NSH // 16) * g2:(NSH // 16) * (g2 + 1), :].rearrange(
                    "(jb p) c -> p jb c", p=_C))

    # reduced (128,1) -> (128, 2): each partition p gets (lo[k], hi[k]) for
    # k = p mod 64; sum folds the halves, duplicated into both halves
    sg2 = consts.tile([_C, 2], f32)
    gv = _bass.AP(tensor=cc_out.tensor, offset=cc_out.offset,
                  ap=[[1, _K], [_K, 2]])
    nc.sync.dma_start(out=sg2[0:_K, :], in_=gv)
    nc.sync.dma_start(out=sg2[_K:_C, :], in_=gv)
    sgsum = consts.tile([_C, 1], f32)
    nc.vector.tensor_reduce(out=sgsum, in_=sg2,
                            axis=mybir.AxisListType.X, op=ALU.add)
    invs = consts.tile([_C, 1], f32)
    nc.vector.reciprocal(out=invs, in_=sgsum)

    # Two zero-padded rhs variants: chunk data lives in one partition half
    # of the paired e_sb; the matmul contracts all 128 rows with the other
    # half zeroed (avoids base_partition-64 bf16 operands, which crash this
    # runtime). augE: rows 0-63 = [Bs^T*invS | invS], rows 64-127 = 0.
    rhs_augE = consts.tile([_C, _C + 1], bf16)
    rhs_augO = consts.tile([_C, _C + 1], bf16)
    nc.vector.memset(rhs_augE[:, :], 0)
    nc.vector.memset(rhs_augO[:, :], 0)
    nc.vector.tensor_scalar_mul(out=rhs_augE[0:_K, 0:_C], in0=B_T[0:_K, :],
                                scalar1=invs[0:_K, :])
    nc.vector.tensor_copy(out=rhs_augE[0:_K, _C:_C + 1], in_=invs[0:_K, :])
    nc.vector.tensor_scalar_mul(out=rhs_augO[_K:_C, 0:_C], in0=B_T[_K:_C, :],
                                scalar1=invs[_K:_C, :])
    nc.vector.tensor_copy(out=rhs_augO[_K:_C, _C:_C + 1], in_=invs[_K:_C, :])

    # ---- phase 2 (software-pipelined emission: MMs+recip+drain for group
    # g, normalize for g-1, relu+store for g-2 — emission order is the Tile
    # scheduler's priority order, which keeps the FIFO engine queues from
    # head-of-line blocking on cross-group dependencies) ----
    SLOTS = [0, 130, 260, 512, 642, 772]
    groups = [list(range(g, min(g + _GB, NBLK))) for g in range(0, NBLK, _GB)]
    NG = len(groups)
    state = {}

    def stage0(gi):
        blocks = groups[gi]
        nb = len(blocks)
        nch = 2 * nb
        pb = ps2.tile([_C, 1024], f32, tag=("pbA", "pbB")[gi % 2])
        slots = SLOTS[:nch] if nb == _GB else [0, 130, 512, 642]
        per_bank = 3 if nb == _GB else 2
        for j, B in enumerate(blocks):
            rhs = rhs_augE if ((B // 2) % 2) == 0 else rhs_augO
            cb = (B // 4) * 512 + 256 * (B % 2)
            for parity in range(2):
                sl = e_sb[0:_C, cb + parity:cb + 256]
                lhsT = _bass.AP(tensor=sl.tensor, offset=sl.offset,
                                ap=[sl.ap[0], [2, _C]])
                nc.tensor.matmul(pb[:, slots[2 * j + parity]:
                                    slots[2 * j + parity] + _C + 1],
                                 lhsT=lhsT, rhs=rhs,
                                 start=True, stop=True)
        gt = _TYPE_PATTERN[gi % len(_TYPE_PATTERN)]
        rq = None
        df = None
        if gt in (0, 1):
            rq = rp.tile([_C, 6], f32, tag="rq")
            d0 = pb[:, _C:_C + 1]
            den = _bass.AP(tensor=d0.tensor, offset=d0.offset,
                           ap=[d0.ap[0], [512, 2], [130, per_bank]])
            nc.vector.reciprocal(out=rq[:, 0:nch], in_=den)
        sa = None
        if gt == 1:
            # A path: ACT drains+scales each chunk straight from PSUM now,
            # so the PSUM tile frees as early as the P path's
            sa = drp.tile([_C, nb, 256], bf16, tag="sa")
            for c in range(nch):
                jb, parity = c // 2, c % 2
                nc.scalar.activation(
                    out=sa[:, jb, parity * _C:(parity + 1) * _C],
                    in_=pb[:, slots[c]:slots[c] + _C],
                    func=AF.Copy, bias=0.0, scale=rq[:, c:c + 1])
        elif gt == 2:
            # P path: f32 drain of zz+denoms, split per bank so Pool's
            # divides start as soon as the first bank's matmuls land
            df = drp.tile([_C, nb, 2, 129], f32, tag="df")
            dflat = df[:, :, :, :].rearrange("p a b c -> p (a b c)")
            for bk in range(2):
                z0 = pb[:, 512 * bk:512 * bk + 1]
                zd = _bass.AP(tensor=z0.tensor, offset=z0.offset,
                              ap=[z0.ap[0], [130, per_bank], [1, 129]])
                nc.scalar.activation(
                    out=dflat[:, bk * per_bank * 129:(bk + 1) * per_bank * 129],
                    in_=zd, func=AF.Copy, bias=0.0, scale=1.0)
        state[gi] = (pb, rq, df, sa, slots, gt)

    # stage tiles span a PAIR of groups (6 blocks) so relu + store run once
    # per pair
    def stage1(gi):
        blocks = groups[gi]
        nb = len(blocks)
        nch = 2 * nb
        pb, rq, df, sa, slots, gt = state[gi]
        qi = gi // 4
        if gi % 4 == 0:
            nbst = sum(len(groups[g]) for g in range(gi, min(gi + 4, NG)))
            st = stp.tile([_C, nbst, 256], bf16, tag="st")
            state[("st", qi)] = st
            joff = 0
        else:
            st = state[("st", qi)]
            joff = sum(len(groups[g]) for g in range(4 * qi, gi))
        if gt == 0:
            for c in range(nch):
                jb, parity = c // 2, c % 2
                nc.vector.scalar_tensor_tensor(
                    out=st[:, joff + jb, parity * _C:(parity + 1) * _C],
                    in0=pb[:, slots[c]:slots[c] + _C],
                    scalar=rq[:, c:c + 1],
                    in1=xst_sb[:, blocks[jb], parity * _C:(parity + 1) * _C],
                    op0=ALU.mult, op1=ALU.add)
        else:
            if gt == 2:
                sa = drp.tile([_C, nb, 256], bf16, tag="sa")
                for c in range(nch):
                    jb, parity = c // 2, c % 2
                    # Pool divides the drained f32 copy by its denominator
                    nc.gpsimd.normalize_recip(
                        out_ap=sa[:, jb, parity * _C:(parity + 1) * _C],
                        in_ap=df[:, jb, parity, 0:_C],
                        denom_ap=df[:, jb, parity, _C:_C + 1])
            nc.vector.tensor_add(
                out=st[:, joff:joff + nb, :], in0=sa,
                in1=xst_sb[:, blocks[0]:blocks[0] + nb, :])
        state.pop(gi)

    def stage2(qi):
        blocks = [b for g in range(4 * qi, min(4 * qi + 4, NG))
                  for b in groups[g]]
        nb = len(blocks)
        st = state.pop(("st", qi))
        nc.vector.tensor_scalar_max(out=st, in0=st, scalar1=0.0)
        B0 = blocks[0]
        nc.sync.dma_start(
            out=out_d[_C * B0:_C * (B0 + nb), :].rearrange(
                "(jb p) c -> p jb c", p=_C),
            in_=st)

    for i in range(NG + 5):
        if i < NG:
            stage0(i)
        g1 = i - 3
        if 0 <= g1 < NG:
            stage1(g1)
            if g1 % 4 == 3 or g1 == NG - 1:
                stage2(g1 // 4)


def _host_fold(inputs):
    f64 = np.float64
    lin0 = np.asarray(inputs["lin0_w"], f64)
    conv1 = np.asarray(inputs["conv1_w"], f64)
    conv1b = np.asarray(inputs["conv1_b"], f64)
    conv2 = np.asarray(inputs["conv2_w"], f64)
    lin1 = np.asarray(inputs["lin1_w"], f64)
    gamma = np.asarray(inputs["bn_gamma"], f64)
    beta = np.asarray(inputs["bn_beta"], f64)
    mean = np.asarray(inputs["bn_mean"], f64)
    var = np.asarray(inputs["bn_var"], f64)

    A = (lin0 @ conv1).astype(np.float32)                       # (64,128)
    ab = (lin0 @ conv1b).astype(np.float32)                     # (64,)
    scale = gamma / np.sqrt(var + _BN_EPS)
    shift = (beta - mean * scale).astype(np.float32)            # (128,)
    Bm = ((scale[:, None] * conv2) @ lin1).astype(np.float32)   # (128,64)
    return A, ab, shift, Bm


def _shard_inputs(inputs):
    import concourse.mybir as mybir
    bf16 = mybir.dt.np(mybir.dt.bfloat16)

    x = np.ascontiguousarray(np.asarray(inputs["x"], dtype=np.float32))
    A, ab, shift, Bm = _host_fold(inputs)

    a_t = np.ascontiguousarray(A.T).astype(bf16)                # (128, 64)
    bt64 = np.ascontiguousarray(Bm.T).astype(bf16)              # (64, 128)
    b_t = np.concatenate([bt64, bt64], axis=0)                  # (128, 128)
    ab2 = np.concatenate([ab, ab]).reshape(_C, 1).astype(np.float32)

    xf = x.reshape(_B, _C, _N)
    in_maps = []
    for g in range(_NCORES):
        b = g // _SLICES
        s = g % _SLICES
        x_sh = np.ascontiguousarray(xf[b, :, s * _NSH:(s + 1) * _NSH])
        xst_sh = (x_sh.T + shift[None, :]).astype(bf16)
        in_maps.append({
            "x": np.ascontiguousarray(x_sh.astype(bf16)),
            "xst": np.ascontiguousarray(xst_sh.reshape(_NSH // 2, 256)),
            "a_t": a_t,
            "b_t": b_t,
            "ab2": ab2,
        })
    return in_maps


def kernel(**inputs):
    global last_results
    import time
    from concourse.bass_utils import run_bass_kernel_spmd

    in_maps = _shard_inputs(inputs)
    nc = _build()
    last_err = None
    for attempt in range(3):
        try:
            last_results = run_bass_kernel_spmd(
                nc, in_maps, core_ids=list(range(_NCORES)))
            break
        except Exception as e:  # transient axon worker hiccups: retry
            last_err = e
            if attempt == 2:
                raise
            time.sleep(20.0 * (attempt + 1))

    full = np.empty((_B, _C, _N), np.float32)
    for g in range(_NCORES):
        b = g // _SLICES
        s = g % _SLICES
        o2 = last_results.results[g]["out"].astype(np.float32)
        full[b, :, s * _NSH:(s + 1) * _NSH] = o2.reshape(_NSH, _C).T
    return full.reshape(_B, _C, _D, _H, _W)

